# revision 30
# baseline (speedup 1.0000x reference)
"""Galerkin attention (ragged graph segments) on 8 Trainium2 NeuronCores — v3.

Math (per reference):
  qkv = x @ w_qkv.T ; split q,k,v -> [B, H, N, DH]
  k, v  <- LayerNorm over DH (eps=1e-6, affine)
  per graph g (sorted contiguous segments of N): ktv[g] = k_g^T v_g
  out_n = (q_n / size(g(n))) @ ktv[g(n)]
  y = out @ w_out.T + b_out

v3 design vs the v2 baseline (which was PE.SEQ + DVE bound):
  * The whole q path is fused into a per-(b,graph) matrix
      M = Wq^T  blockdiag(ktv)/ng  Wo^T           [DIM, DIM]
    built once per graph (20 free-512 matmuls), so each node tile needs
    only y = x_tile @ M (4 matmuls) instead of q-proj + oh + out-proj
    (9 matmuls + 2 ACT staging copies).  LN weights w1/w2 fold into the
    host-side Wq/Wo constants; 1/ng folds into the bd assembly.
  * LayerNorm is applied to k only:  k* = (alpha*beta) k + (beta*gamma);
    v enters the Gram RAW.  Identity:
      ktv = sum k' v'^T = sum k* v^T - (sum mu_v k*) 1^T
    The correction column is produced by the SAME Gram matmul: the v tile
    is staged as [pair, 130] with mu_v appended per head, so each pair's
    Gram+correction is ONE free-130 matmul.
  * LN stats via one multi-group bn_stats per side (DVE) + small even/odd
    combine ops (GPSIMD); the k apply is 8 per-head tensor_scalar ops that
    hit the DVE 4x_2p fast path (fp16, per-partition scalars).
  * Output tiles DMA straight from PSUM (f32), no staging copy.
"""

import os
import sys

if "/opt/trn_rl_repo" not in sys.path:
    sys.path.insert(0, "/opt/trn_rl_repo")

import numpy as np

import concourse.bacc as bacc
import concourse.bass as bass
import concourse.mybir as mybir
import concourse.tile as tile
from concourse.bass_utils import run_bass_kernel_spmd

P = 128
B = 2
DIM = 512
HEADS = 8
DH = 64
INNER = HEADS * DH          # 512
NCH = DIM // P              # 4 contraction chunks
NPAIRS = HEADS // 2         # 4 head pairs
DA2 = 2 * DH + 2            # pair Gram free width: 2 heads + 2 mu_v cols
EPS = 1e-6
N_CORES = 8
F32 = mybir.dt.float32
F16 = mybir.dt.float16

_PROGRAM_CACHE: dict = {}


def _apv(t, poff, pcnt, foff, dims):
    """Manual AP view of tile t: partition range [poff, poff+pcnt), free
    offset foff, free dims [[stride, n], ...] (strides in elements)."""
    a = t[:]
    ps = a.ap[0][0]
    return bass.AP(a.tensor, a.offset + poff * ps + foff,
                   [[ps, pcnt]] + [list(d) for d in dims])


def _groups(L, grp=4):
    if L <= grp:
        return [(0, L)]
    out = []
    t0 = 0
    while L - t0 > grp + 1:
        out.append((t0, grp))
        t0 += grp
    rem = L - t0
    if rem > grp:  # rem == grp + 1
        out.append((t0, grp - 1))
        out.append((t0 + grp - 1, 2))
    else:
        out.append((t0, rem))
    return out


# ---------------------------------------------------------------------------
# host-side planning
# ---------------------------------------------------------------------------

def _plan(batch, num_graphs, n_cores):
    batch = np.asarray(batch).astype(np.int64)
    G = int(num_graphs)
    counts = np.bincount(batch, minlength=G)[:G].astype(np.int64)
    starts = np.concatenate([[0], np.cumsum(counts)[:-1]])
    tiles_g = (counts + P - 1) // P

    S = (G + n_cores - 1) // n_cores
    order = np.argsort(-tiles_g, kind="stable")
    core_graphs = [[] for _ in range(n_cores)]
    core_load = [0] * n_cores
    for g in order:
        cands = [c for c in range(n_cores) if len(core_graphs[c]) < S]
        c = min(cands, key=lambda cc: (core_load[cc], cc))
        core_graphs[c].append(int(g))
        core_load[c] += int(tiles_g[g])
    for c in range(n_cores):
        core_graphs[c].sort(key=lambda g: -int(tiles_g[g]))
        while len(core_graphs[c]) < S:
            core_graphs[c].append(-1)

    Ls = []
    for s in range(S):
        L = max(
            int(tiles_g[core_graphs[c][s]]) if core_graphs[c][s] >= 0 else 0
            for c in range(n_cores)
        )
        Ls.append(max(L, 1))
    return counts, starts, core_graphs, Ls


def _pack_inputs(x, counts, starts, core_graphs, Ls, n_cores):
    """Pack per-core inputs slot-blocked for single-DMA-per-slot transfers.

    xblk layout per (b, slot): [c=128 partitions, NCH, L*P] flattened at
    element offset b*T*P*DIM + DIM*P*slot_off[s]; per partition the run is
    contiguous (NCH*L*P elems)."""
    T = sum(Ls)
    slot_off = np.concatenate([[0], np.cumsum(Ls)[:-1]])
    xT = np.ascontiguousarray(
        np.transpose(x, (0, 2, 1)).astype(np.float16))  # [B, DIM, N] fp16
    per_core = []
    for c in range(n_cores):
        xTp = np.zeros((B, DIM, T * P), np.float16)
        gsc = np.zeros((T * P,), np.float32)
        for s, g in enumerate(core_graphs[c]):
            if g < 0 or counts[g] == 0:
                continue
            n0, ng = int(starts[g]), int(counts[g])
            off = int(slot_off[s]) * P
            xTp[:, :, off:off + ng] = xT[:, :, n0:n0 + ng]
            # per-slot 1/ng on EVERY row of the slot's tiles (bd scale)
            L = int(Ls[s])
            gsc[off:off + L * P] = 1.0 / ng
        # slot-block: [DIM, T*P] -> per slot [128, NCH, L*P]
        xv = xTp.reshape(B, NCH, P, T * P)
        blk = np.empty((B, T * P * DIM), np.float16)
        for s in range(len(Ls)):
            off = int(slot_off[s])
            L = int(Ls[s])
            seg = xv[:, :, :, off * P:(off + L) * P]          # [B,NCH,P,LP]
            blk[:, DIM * P * off:DIM * P * (off + L)] = (
                seg.transpose(0, 2, 1, 3).reshape(B, -1))
        per_core.append((blk, gsc))
    return per_core, slot_off


# ---------------------------------------------------------------------------
# device program
# ---------------------------------------------------------------------------

def _build_program(T, Ls, n_cores, bo_zero=True, repeat=1, skeleton=0):
    from contextlib import ExitStack

    nc = bacc.Bacc("TRN2", target_bir_lowering=False, debug=False,
                   num_devices=n_cores)

    xT = nc.dram_tensor("xT", [B, T * P * DIM], F16, kind="ExternalInput")
    wkv = nc.dram_tensor("wkvT", [DIM, 2 * INNER], F16, kind="ExternalInput")
    wqp = nc.dram_tensor("wqp", [P, NPAIRS, DIM], F16, kind="ExternalInput")
    wop = nc.dram_tensor("wop", [P, NPAIRS, DIM], F16, kind="ExternalInput")
    gsc = nc.dram_tensor("gsc", [T * P], F32, kind="ExternalInput")
    if not bo_zero:
        bo = nc.dram_tensor("bout", [DIM], F32, kind="ExternalInput")
    out = nc.dram_tensor("out", [B, T * P * DIM], F16, kind="ExternalOutput")

    slot_off = [0]
    for L in Ls[:-1]:
        slot_off.append(slot_off[-1] + L)

    Sqrt = mybir.ActivationFunctionType.Sqrt
    mult = mybir.AluOpType.mult
    add = mybir.AluOpType.add
    sub = mybir.AluOpType.subtract

    with ExitStack() as ctx:
        tc = ctx.enter_context(tile.TileContext(nc))
        const = ctx.enter_context(tc.tile_pool(name="const", bufs=1))

        WKV = const.tile([P, NCH, 2 * INNER], F16, tag="WKV")
        nc.sync.dma_start(out=WKV[:],
                          in_=wkv.ap().rearrange("(k c) r -> c k r", c=P))
        WQP = const.tile([P, NPAIRS, DIM], F16, tag="WQP")
        nc.sync.dma_start(out=WQP[:], in_=wqp.ap())
        WOP = const.tile([P, NPAIRS, DIM], F16, tag="WOP")
        nc.sync.dma_start(out=WOP[:], in_=wop.ap())
        GS = const.tile([P, T], F32, tag="GS")
        nc.sync.dma_start(out=GS[:], in_=gsc.ap().rearrange("(t p) -> p t", p=P))
        EPSC = const.tile([P, 1], F32, tag="EPSC")
        nc.vector.memset(EPSC[:], EPS)
        if not bo_zero:
            BOt = const.tile([P, DIM], F32, tag="BO")
            nc.sync.dma_start(out=BOt[:], in_=bo.ap().partition_broadcast(P))

        xpool = ctx.enter_context(tc.tile_pool(name="xp", bufs=3))
        vpool = ctx.enter_context(tc.tile_pool(name="va", bufs=5))
        kstp = ctx.enter_context(tc.tile_pool(name="kst", bufs=5))
        stat = ctx.enter_context(tc.tile_pool(name="stat", bufs=3))
        bdsb = ctx.enter_context(tc.tile_pool(name="bd", bufs=2))
        atsb = ctx.enter_context(tc.tile_pool(name="ats", bufs=5))
        msb = ctx.enter_context(tc.tile_pool(name="ms", bufs=2))
        osb = ctx.enter_context(tc.tile_pool(name="osb", bufs=3))

        kvps = ctx.enter_context(tc.tile_pool(name="kvps", bufs=2, space="PSUM"))
        gps = ctx.enter_context(tc.tile_pool(name="gps", bufs=1, space="PSUM"))
        pmix = ctx.enter_context(tc.tile_pool(name="pmix", bufs=2, space="PSUM"))

        def _emit_mbuild_y(job):
            b2, soff2, L2, xt2, bd2 = job
            # --- M = Wq'^T bd Wo'^T  (w1/w2 folded into WQP/WOP rows) ---
            ats = []
            for p in range(NPAIRS):
                ap_ = pmix.tile([P, DIM], F32, tag="pm")
                nc.tensor.matmul(ap_[:], lhsT=bd2[:, p, :], rhs=WQP[:, p, :],
                                 start=True, stop=True)
                at = atsb.tile([P, DIM], F16, name=f"at{p}", tag="at")
                nc.scalar.copy(out=at[:], in_=ap_[:])
                ats.append(at)
            MS = msb.tile([P, NCH, DIM], F16, tag="ms")
            for c in range(NCH):
                mc = pmix.tile([P, DIM], F32, tag="pm")
                for p in range(NPAIRS):
                    nc.tensor.matmul(
                        mc[:], lhsT=ats[p][:, c * P:(c + 1) * P],
                        rhs=WOP[:, p, :],
                        start=(p == 0), stop=(p == NPAIRS - 1))
                nc.scalar.copy(out=MS[:, c, :], in_=mc[:])

            # --- y = x_tile @ M per tile into the slot slab, one DMA ---
            ot = osb.tile([P, L2, DIM], F16, tag="ot")
            for tl in range(L2):
                yp = pmix.tile([P, DIM], F32, tag="pm")
                for c in range(NCH):
                    nc.tensor.matmul(
                        yp[:], lhsT=xt2[:, c, tl * P:(tl + 1) * P],
                        rhs=MS[:, c, :],
                        start=(c == 0), stop=(c == NCH - 1))
                if bo_zero:
                    nc.scalar.copy(out=ot[:, tl, :], in_=yp[:])
                else:
                    nc.vector.tensor_tensor(ot[:, tl, :], yp[:], BOt[:],
                                            op=add)
            oap = out.ap()[b2]
            dst = bass.AP(oap.tensor, oap.offset + DIM * P * soff2,
                          [[L2 * DIM, P], [DIM, L2], [1, DIM]])
            nc.sync.dma_start(out=dst, in_=ot[:])

        seq = [(b, s) for _rep in range(repeat) for b in range(B)
               for s in range(len(Ls))]
        prev_job = None
        pre_x = None  # prefetched group-0 x tile for the next slot
        for idx, (b, s) in enumerate(seq):
            if True:
                L = Ls[s]
                soff = slot_off[s]

                # Gram accumulators: two PSUM banks of 2 pairs each.
                GA = gps.tile([P, 2, DA2], F32, tag="geoA")
                GB = gps.tile([P, 2, DA2], F32, tag="geoB")
                geo = (GA, GA, GB, GB)

                first = [True]

                def _emit_gram(KST_, KVC_, MUV_):
                    st = first[0]
                    first[0] = False
                    for p in range(NPAIRS):
                        lhs = KST_[:, 2 * p:2 * p + 2, :]
                        nc.tensor.matmul(
                            geo[p][:, p % 2, 0:2 * DH],
                            lhsT=lhs,
                            rhs=KVC_[:, 1, 2 * p:2 * p + 2, :],
                            start=(st and p % 2 == 0), stop=False,
                            skip_group_check=True)
                        nc.tensor.matmul(
                            geo[p][:, p % 2, 2 * DH:2 * DH + 2],
                            lhsT=lhs,
                            rhs=MUV_[:, 2 * p:2 * p + 2],
                            start=False, stop=False,
                            skip_group_check=True)

                pend = []

                def _fetch_x(b_, soff_, L_):
                    LP_ = L_ * P
                    xt_ = xpool.tile([P, NCH, LP_], F16, tag="xt")
                    xap = xT.ap()[b_]
                    src = bass.AP(xap.tensor, xap.offset + DIM * P * soff_,
                                  [[NCH * LP_, P], [LP_, NCH], [1, LP_]])
                    nc.sync.dma_start(out=xt_[:], in_=src)
                    return xt_

                if pre_x is not None:
                    xt = pre_x
                    pre_x = None
                else:
                    xt = _fetch_x(b, soff, L)

                if True:
                    for tl in range(L):
                        # ---- kv projection ----
                        kv = kvps.tile([P, 2, HEADS, DH], F32, tag="kv")
                        for k in range(NCH):
                            lx = xt[:, k, tl * P:(tl + 1) * P]
                            nc.tensor.matmul(
                                kv[:, 0], lhsT=lx, rhs=WKV[:, k, 0:INNER],
                                start=(k == 0), stop=(k == NCH - 1))
                            nc.tensor.matmul(
                                kv[:, 1], lhsT=lx, rhs=WKV[:, k, INNER:],
                                start=(k == 0), stop=(k == NCH - 1))
                        if len(pend) >= 3:
                            _emit_gram(*pend.pop(0))

                        # ---- stage k|v fp16 (plain [side, head, DH]) ----
                        KVC = vpool.tile([P, 2, HEADS, DH], F16, tag="kvc")
                        nc.scalar.copy(out=KVC[:], in_=kv[:])

                        if skeleton:
                            # timing skeleton: skip LN chain (numerically wrong)
                            KSTs = kstp.tile([P, HEADS, DH], F16, tag="kst")
                            MUVs = stat.tile([P, HEADS], F16, tag="muv")
                            nc.vector.tensor_copy(KSTs[:], KVC[:, 0])
                            nc.vector.tensor_copy(
                                MUVs[:],
                                _apv(KVC, 0, P, INNER, [[DH, HEADS]]))
                            pend.append((KSTs, KVC, MUVs))
                            if tl == min(3, L - 1):
                                if prev_job is not None:
                                    _emit_mbuild_y(prev_job)
                                    prev_job = None
                                if idx + 1 < len(seq):
                                    nb, ns = seq[idx + 1]
                                    pre_x = _fetch_x(nb, slot_off[ns], Ls[ns])
                            continue
                        # ---- stats: SU = sum, SS = sum of squares per head
                        SQT = vpool.tile([P, 2, HEADS, DH], F16, tag="sqt")
                        nc.gpsimd.tensor_tensor(SQT[:], KVC[:], KVC[:],
                                                op=mult)
                        SU = stat.tile([P, 2, HEADS], F32, tag="su")
                        nc.vector.tensor_reduce(
                            SU[:], KVC[:], axis=mybir.AxisListType.X, op=add)
                        SS = stat.tile([P, 2, HEADS], F32, tag="ss")
                        nc.vector.tensor_reduce(
                            SS[:], SQT[:], axis=mybir.AxisListType.X, op=add)
                        # vt = SS - SU^2/64 ; sd = sqrt(vt/64 + eps)
                        MUSQ = stat.tile([P, 2, HEADS], F32, tag="musq")
                        nc.gpsimd.tensor_tensor(MUSQ[:], SU[:], SU[:], op=mult)
                        VT = stat.tile([P, 2, HEADS], F32, tag="vt")
                        nc.vector.scalar_tensor_tensor(
                            VT[:], MUSQ[:], -1.0 / DH, SS[:], op0=mult, op1=add)
                        SD = stat.tile([P, 2, HEADS], F32, tag="sd")
                        nc.scalar.activation(SD[:], VT[:], Sqrt,
                                             bias=EPSC[:, 0:1], scale=1.0 / DH)
                        RSD = stat.tile([P, 2, HEADS], F32, tag="rsd")
                        nc.vector.reciprocal(RSD[:], SD[:])
                        # coefs: AB = alpha*beta ; BG = -mu_k*AB ; MUV = mu_v
                        AB = stat.tile([P, HEADS], F32, tag="ab")
                        nc.gpsimd.tensor_tensor(AB[:], RSD[:, 0], RSD[:, 1],
                                                op=mult)
                        BG = stat.tile([P, HEADS], F32, tag="bg")
                        nc.vector.scalar_tensor_tensor(
                            BG[:], SU[:, 0], -1.0 / DH, AB[:],
                            op0=mult, op1=mult)
                        MUV = stat.tile([P, HEADS], F16, tag="muv")
                        nc.vector.tensor_scalar(MUV[:], SU[:, 1], 1.0 / DH,
                                                None, op0=mult)

                        # ---- k* = AB*k + BG (per-head, DVE 4x path) ----
                        KST = kstp.tile([P, HEADS, DH], F16, tag="kst")
                        for h in range(HEADS):
                            nc.vector.tensor_scalar(
                                KST[:, h], KVC[:, 0, h],
                                AB[:, h:h + 1], BG[:, h:h + 1],
                                op0=mult, op1=add)
                        pend.append((KST, KVC, MUV))

                        if tl == min(3, L - 1):
                            # previous slot's M-build + y fills the PE while
                            # this slot's LN chains run; prefetch next slot's
                            # x behind it.
                            if prev_job is not None:
                                _emit_mbuild_y(prev_job)
                                prev_job = None
                            if idx + 1 < len(seq):
                                nb, ns = seq[idx + 1]
                                pre_x = _fetch_x(nb, slot_off[ns], Ls[ns])

                while pend:
                    _emit_gram(*pend.pop(0))

                # ---- bd = (G - ctilde 1^T) / ng, pair block-diagonal ----
                bd = bdsb.tile([P, NPAIRS, P], F16, tag="bd")
                nc.gpsimd.memset(bd[:], 0.0)
                for p in range(NPAIRS):
                    Gt = geo[p]
                    pp = p % 2
                    nc.vector.tensor_scalar(
                        bd[0:DH, p, 0:DH], Gt[0:DH, pp, 0:DH],
                        Gt[0:DH, pp, 2 * DH:2 * DH + 1],
                        _apv(GS, 0, DH, soff, [[1, 1]]),
                        op0=sub, op1=mult)
                    nc.vector.tensor_scalar(
                        bd[DH:P, p, DH:P], Gt[DH:P, pp, DH:2 * DH],
                        Gt[DH:P, pp, 2 * DH + 1:2 * DH + 2],
                        _apv(GS, DH, DH, soff, [[1, 1]]),
                        op0=sub, op1=mult)

                prev_job = (b, soff, L, xt, bd)

        if prev_job is not None:
            _emit_mbuild_y(prev_job)

    nc.compile()
    return nc


# ---------------------------------------------------------------------------
# entry point
# ---------------------------------------------------------------------------

def _host_fallback(x, w_qkv, ln1_w, ln1_b, ln2_w, ln2_b, w_out, b_out,
                   batch, num_graphs):
    """Slow exact numpy path (only for LN biases != 0, never produced by
    the reference's setup_inputs)."""
    d = np.float64
    Bx, N, DIMx = x.shape
    qkv = x.astype(d) @ np.asarray(w_qkv, d).T
    q, k, v = np.split(qkv, 3, axis=-1)
    to_bhnd = lambda t: t.reshape(Bx, N, HEADS, DH).transpose(0, 2, 1, 3)
    q, k, v = to_bhnd(q), to_bhnd(k), to_bhnd(v)

    def ln(t, w, b_):
        mu = t.mean(-1, keepdims=True)
        var = ((t - mu) ** 2).mean(-1, keepdims=True)
        return (t - mu) / np.sqrt(var + EPS) * np.asarray(w, d) + np.asarray(b_, d)

    k = ln(k, ln1_w, ln1_b)
    v = ln(v, ln2_w, ln2_b)
    batch = np.asarray(batch).astype(np.int64)
    sizes = np.bincount(batch, minlength=int(num_graphs))
    qn = q / np.maximum(sizes, 1)[batch][None, None, :, None]
    outv = np.zeros_like(qn)
    for g in range(int(num_graphs)):
        m = batch == g
        if not m.any():
            continue
        ktv = np.einsum("bhnd,bhne->bhde", k[:, :, m], v[:, :, m])
        outv[:, :, m] = np.einsum("bhnd,bhde->bhne", qn[:, :, m], ktv)
    outv = outv.transpose(0, 2, 1, 3).reshape(Bx, N, INNER)
    return (outv @ np.asarray(w_out, d).T + np.asarray(b_out, d)).astype(
        np.float32)


def _run(x, w_qkv, ln1_w, ln1_b, ln2_w, ln2_b, w_out, b_out, batch,
         num_graphs, n_cores=N_CORES, trace=False):
    x = np.asarray(x)
    ln1_b = np.asarray(ln1_b, np.float32)
    ln2_b = np.asarray(ln2_b, np.float32)
    if np.any(ln1_b != 0.0) or np.any(ln2_b != 0.0):
        return _host_fallback(x, w_qkv, ln1_w, ln1_b, ln2_w, ln2_b, w_out,
                              b_out, batch, num_graphs), None

    counts, starts, core_graphs, Ls = _plan(batch, num_graphs, n_cores)
    per_core, slot_off = _pack_inputs(x, counts, starts, core_graphs, Ls,
                                      n_cores)
    T = sum(Ls)

    bout_np = np.asarray(b_out, np.float32)
    bo_zero = bool(np.all(bout_np == 0.0))
    key = (T, tuple(Ls), n_cores, bo_zero)
    nc = _PROGRAM_CACHE.get(key)
    if nc is None:
        nc = _build_program(T, tuple(Ls), n_cores, bo_zero)
        _PROGRAM_CACHE[key] = nc

    w1 = np.asarray(ln1_w, np.float32)
    w2 = np.asarray(ln2_w, np.float32)
    wq = np.asarray(w_qkv, np.float32)[:INNER]          # [INNER, DIM]
    wkvT = np.ascontiguousarray(
        np.asarray(w_qkv, np.float32)[INNER:].T.astype(np.float16))
    # pair-stacked Wq with w1 folded into k-dims (ktv rows)
    wqp_np = (wq.reshape(NPAIRS, P, DIM)
              * np.tile(w1, 2)[None, :, None]).astype(np.float16)
    wqp_np = np.ascontiguousarray(wqp_np.transpose(1, 0, 2))  # [P, NPAIRS, DIM]
    # pair-stacked Wo^T with w2 folded into v-dims (ktv cols)
    woT = np.asarray(w_out, np.float32).T                # [INNER, DIM]
    wop_np = (woT.reshape(NPAIRS, P, DIM)
              * np.tile(w2, 2)[None, :, None]).astype(np.float16)
    wop_np = np.ascontiguousarray(wop_np.transpose(1, 0, 2))

    in_maps = []
    for c in range(n_cores):
        xTp, gscv = per_core[c]
        m = {"xT": xTp, "wkvT": wkvT, "wqp": wqp_np, "wop": wop_np,
             "gsc": gscv}
        if not bo_zero:
            m["bout"] = np.ascontiguousarray(bout_np)
        in_maps.append(m)

    res = run_bass_kernel_spmd(nc, in_maps, list(range(n_cores)), trace=trace)

    N = x.shape[1]
    y = np.empty((B, N, DIM), np.float32)
    for c in range(n_cores):
        oc = res.results[c]["out"]        # [B, T*P*DIM] slot-blocked
        for s, g in enumerate(core_graphs[c]):
            if g < 0 or counts[g] == 0:
                continue
            n0, ng = int(starts[g]), int(counts[g])
            off = int(slot_off[s])
            L = int(Ls[s])
            seg = oc[:, DIM * P * off:DIM * P * (off + L)]
            seg = seg.reshape(B, P, L, DIM).transpose(0, 2, 1, 3)
            y[:, n0:n0 + ng, :] = seg.reshape(B, L * P, DIM)[:, :ng].astype(
                np.float32)
    return y, res


def kernel(**inputs):
    trace = os.environ.get("GALERKIN_TRACE") == "1"
    y, _ = _run(
        inputs["x"], inputs["w_qkv"], inputs["ln1_w"], inputs["ln1_b"],
        inputs["ln2_w"], inputs["ln2_b"], inputs["w_out"], inputs["b_out"],
        inputs["batch"], inputs["num_graphs"], trace=trace,
    )
    return y
